# revision 28
# baseline (speedup 1.0000x reference)
"""GCN encoder (2-layer masked-attention message passing) on 8 Trainium2 cores.

Sharding: data-parallel over batch B=8 -> 1 graph per NeuronCore. Each core
holds the full (small) parameter set and its own [N, N] edge-mask slab.

Per-core algorithm (N=2048 nodes, E=512, FF=256, 2 layers), all on-chip:
  - x kept in SBUF in natural [n, E] layout (residual/LN) and transposed
    [E, n] layout (projection rhs).
  - Q^T/K^T = W x^T computed with E-contraction on partitions.
  - scores are computed TRANSPOSED: sT[i, o] = sum_f K^T[f,i] Q^T[f,o], so the
    unnormalized attention p = mask * exp(sT/16) is already in the [i, o]
    layout needed as the moving operand of the ctx matmul (no O(N^2) on-chip
    transpose). Softmax max-subtraction is skipped: scores are ~N(0, 0.2^2)
    by construction, exp() cannot overflow, and softmax is shift-invariant.
  - denominator: ones[128,128]^T @ p broadcasts column sums to every PSUM
    partition; normalization (x 1/denom) is fused into the ctx PSUM->SBUF
    copy, so it costs no extra DVE pass over the N^2 matrix.
  - ctx^T[d, o] accumulated over node chunks, then ctx2 = ctx @ Wc^T lands
    back in natural [o, e] layout for residual + LayerNorm (bn_stats/bn_aggr,
    rstd via DVE Newton so the ACT Exp LUT is never evicted).
  - all matmul operands use float32r (fp32 bits, PE streams 1 row/cycle
    instead of fp32's 4 -> 4x matmul throughput, ~1.5e-4 rel err).
"""

import os
import sys

for _p in ("/root/.axon_site/_ro/trn_rl_repo", "/opt/trn_rl_repo"):
    if os.path.isdir(_p) and _p not in sys.path:
        sys.path.append(_p)

import numpy as np

B, N, E, FF, L = 8, 2048, 512, 256, 2
P = 128
NC = N // P  # 16 node chunks
EC = E // P  # 4 embed chunks
FC = FF // P  # 2 ff chunks
OW = 512  # o-tile width (out-node tile)
OT = N // OW  # 4 o-tiles
OS = OW // P  # 4 o-subtiles per o-tile
INV_SCALE = 1.0 / float(np.sqrt(FF))
LN_EPS = 1e-5

_CACHE = {}


def _build(apply_gb: bool):
    import concourse.bass as bass
    import concourse.tile as tile
    from concourse import bacc, mybir

    f32 = mybir.dt.float32
    f32r = mybir.dt.float32r
    i32 = mybir.dt.int32
    AF = mybir.ActivationFunctionType
    ALU = mybir.AluOpType

    nc = bacc.Bacc(
        "TRN2", target_bir_lowering=False, debug=False, num_devices=B
    )

    x0 = nc.dram_tensor("x0", [N, E], f32r, kind="ExternalInput")
    x0t = nc.dram_tensor("x0t", [E, N], f32r, kind="ExternalInput")
    # edge mask, transposed: maskt[i, o] = 1.0 if edge(o <- i) else 0.0
    maskt = nc.dram_tensor("maskt", [N, N], f32r, kind="ExternalInput")
    wq = nc.dram_tensor("wq", [L, E, FF], f32r, kind="ExternalInput")  # Wq^T
    wk = nc.dram_tensor("wk", [L, E, FF], f32r, kind="ExternalInput")  # Wk^T
    wc = nc.dram_tensor("wc", [L, E, E], f32r, kind="ExternalInput")  # Wc^T
    idn = nc.dram_tensor("idn", [P, P], f32r, kind="ExternalInput")
    onesm = nc.dram_tensor("onesm", [P, P], f32r, kind="ExternalInput")
    bq = nc.dram_tensor("bq", [L, FF], f32, kind="ExternalInput")
    bk = nc.dram_tensor("bk", [L, FF], f32, kind="ExternalInput")
    if apply_gb:
        ln_g = nc.dram_tensor("ln_g", [L, E], f32, kind="ExternalInput")
        ln_b = nc.dram_tensor("ln_b", [L, E], f32, kind="ExternalInput")
    out = nc.dram_tensor("out", [N, E], f32, kind="ExternalOutput")

    with tile.TileContext(nc) as tc:
        with (
            tc.tile_pool(name="persist", bufs=1) as persist,
            tc.tile_pool(name="wpool", bufs=1) as wpool,
            tc.tile_pool(name="qt", bufs=1) as qtpool,
            tc.tile_pool(name="pt", bufs=1) as ptpool,
            tc.tile_pool(name="ctxt", bufs=1) as ctxtpool,
            tc.tile_pool(name="mask", bufs=6 if not apply_gb else 4) as mpool,
            tc.tile_pool(name="rb", bufs=2) as rbpool,
            tc.tile_pool(name="ln", bufs=8) as lnpool,
            tc.tile_pool(name="ps", bufs=4, space="PSUM") as pspool,
            tc.tile_pool(name="psdn", bufs=2, space="PSUM") as dnpool,
            tc.tile_pool(name="pstr", bufs=2, space="PSUM") as trpool,
        ):
            # ---------------- persistent tiles ----------------
            XA = persist.tile([P, NC, E], f32r, tag="XA")  # layer-1 input
            XB = persist.tile([P, NC, E], f32r, tag="XB")  # layer-1 output
            XT = persist.tile([P, EC, N], f32r, tag="XT")  # x^T of current layer
            KT = persist.tile([P, FC, N], f32r, tag="KT")
            ident = persist.tile([P, P], f32r, tag="ident")
            ones = persist.tile([P, P], f32r, tag="ones")
            bq_sb = persist.tile([P, L, FC], f32, tag="bq")
            bk_sb = persist.tile([P, L, FC], f32, tag="bk")

            def bcast_ap(src):  # broadcast a DRAM AP across all partitions
                return bass.AP(
                    tensor=src.tensor, offset=src.offset, ap=[[0, P], *src.ap]
                )

            # DMA issue order matters: HWDGE queues drain in issue order, so
            # put the first-consumed tensors (layer-0 K/Q weights, XT) first.
            nc.sync.dma_start(out=ident, in_=idn[:, :])
            nc.sync.dma_start(out=ones, in_=onesm[:, :])
            nc.sync.dma_start(
                out=bq_sb, in_=bq.rearrange("l (c p) -> p l c", p=P)
            )
            nc.sync.dma_start(
                out=bk_sb, in_=bk.rearrange("l (c p) -> p l c", p=P)
            )
            g_sb = b_sb = None
            if apply_gb:
                g_sb = persist.tile([P, L, E], f32, tag="g")
                b_sb = persist.tile([P, L, E], f32, tag="b")
                nc.gpsimd.dma_start(out=g_sb, in_=bcast_ap(ln_g[:, :]))
                nc.gpsimd.dma_start(out=b_sb, in_=bcast_ap(ln_b[:, :]))

            def load_weights(k, split=False):
                wq_sb = wpool.tile([P, EC, FF], f32r, tag="wq")
                wk_sb = wpool.tile([P, EC, FF], f32r, tag="wk")
                wc_sb = wpool.tile([P, EC, E], f32r, tag="wc")
                nc.sync.dma_start(
                    out=wk_sb, in_=wk[k].rearrange("(c p) f -> p c f", p=P)
                )
                if not split:
                    nc.sync.dma_start(
                        out=wq_sb, in_=wq[k].rearrange("(c p) f -> p c f", p=P)
                    )
                    nc.sync.dma_start(
                        out=wc_sb, in_=wc[k].rearrange("(c p) e -> p c e", p=P)
                    )
                return wq_sb, wk_sb, wc_sb

            # issue order: wk -> XT(nt0) -> rest of XT -> wq -> wc, so the
            # first K-projection matmul starts after ~1.5MB instead of 10MB
            w0 = load_weights(0, split=True)
            x0tr = x0t.rearrange("(c p) n -> p c n", p=P)
            for nt in range(OT):
                for ec in range(EC):
                    nc.sync.dma_start(
                        out=XT[:, ec, nt * OW : (nt + 1) * OW],
                        in_=x0tr[:, ec, nt * OW : (nt + 1) * OW],
                    )
                if nt == 0:
                    nc.sync.dma_start(
                        out=w0[0], in_=wq[0].rearrange("(c p) f -> p c f", p=P)
                    )
            nc.sync.dma_start(
                out=w0[2], in_=wc[0].rearrange("(c p) e -> p c e", p=P)
            )
            x0r = x0.rearrange("(c p) e -> p c e", p=P)

            w_next = w0
            for k in range(L):
                X_in = XA if k == 0 else XB

                wq_sb, wk_sb, wc_sb = w_next

                if k > 0:
                    # rebuild XT from X_in via PE transposes (late node
                    # chunks last, so PE isn't blocked on layer-1's tail)
                    for ncn in range(NC):
                        for ec in range(EC):
                            pst = trpool.tile([P, P], f32r, tag="tr")
                            nc.tensor.transpose(
                                pst,
                                X_in[:, ncn, ec * P : (ec + 1) * P],
                                ident,
                            )
                            nc.scalar.copy(
                                XT[:, ec, ncn * P : (ncn + 1) * P], pst
                            )

                # K^T and Q^T projections for the whole layer
                QT = qtpool.tile([P, FC, N], f32r, tag="qt")
                for dst, w_sb, b_sb2 in ((KT, wk_sb, bk_sb), (QT, wq_sb, bq_sb)):
                    for nt in range(OT):
                        for fc in range(FC):
                            ps = pspool.tile([P, OW], f32, tag="mm")
                            for ec in range(EC):
                                nc.tensor.matmul(
                                    ps,
                                    lhsT=w_sb[:, ec, fc * P : (fc + 1) * P],
                                    rhs=XT[:, ec, nt * OW : (nt + 1) * OW],
                                    start=(ec == 0),
                                    stop=(ec == EC - 1),
                                )
                            nc.scalar.add(
                                dst[:, fc, nt * OW : (nt + 1) * OW],
                                ps,
                                b_sb2[:, k, fc : fc + 1],
                            )

                # queue next layer's weight loads now: wk/wq slots are free
                # after the projections above, so the DMAs overlap this
                # layer's attention phases instead of stalling layer k+1
                if k + 1 < L:
                    w_next = load_weights(k + 1)

                for ot in range(OT):
                    osl = slice(ot * OW, (ot + 1) * OW)
                    QTt = QT[:, :, osl]

                    # scores^T + exp + mask -> pT[i, o]; denom via ones-matmul
                    pT = ptpool.tile([P, NC, OW], f32r, tag="pT")
                    dn = dnpool.tile([P, OW], f32, tag="dn")
                    for ic in range(NC):
                        ps = pspool.tile([P, OW], f32, tag="mm")
                        for fc in range(FC):
                            nc.tensor.matmul(
                                ps,
                                lhsT=KT[:, fc, ic * P : (ic + 1) * P],
                                rhs=QTt[:, fc, :],
                                start=(fc == 0),
                                stop=(fc == FC - 1),
                            )
                        nc.scalar.activation(
                            pT[:, ic, :], ps, AF.Exp, scale=INV_SCALE
                        )
                        mt = mpool.tile([P, OW], f32r, tag="mt")
                        nc.sync.dma_start(
                            mt, maskt[ic * P : (ic + 1) * P, osl]
                        )
                        nc.vector.tensor_mul(pT[:, ic, :], pT[:, ic, :], mt)
                        # dn[p, o] = column sums of p, broadcast to all partitions
                        nc.tensor.matmul(
                            dn,
                            lhsT=ones,
                            rhs=pT[:, ic, :],
                            start=(ic == 0),
                            stop=(ic == NC - 1),
                        )
                        if k == 0 and ot == 0:
                            # stream layer-0 x in behind the first masks;
                            # it is first needed by phase C of o-tile 0
                            nc.sync.dma_start(
                                out=XA[:, ic, :], in_=x0r[:, ic, :]
                            )
                    rb = rbpool.tile([P, OW], f32, tag="rb")
                    nc.vector.reciprocal(rb, dn)

                    # ctx^T[d, o] = sum_i x[i, d] * p[i, o], normalized by rb
                    ctxT = ctxtpool.tile([P, EC, OW], f32r, tag="ctxT")
                    for ec in range(EC):
                        ps = pspool.tile([P, OW], f32, tag="mm")
                        for ic in range(NC):
                            nc.tensor.matmul(
                                ps,
                                lhsT=X_in[:, ic, ec * P : (ec + 1) * P],
                                rhs=pT[:, ic, :],
                                start=(ic == 0),
                                stop=(ic == NC - 1),
                            )
                        nc.vector.tensor_mul(ctxT[:, ec, :], ps, rb)

                    # ctx2 = ctx @ Wc^T, residual, LayerNorm.
                    # Layer-1 output lives in XB; layer-2 h reuses XA (dead
                    # in layer 2). LN stats for the 4 o-subtiles are batched
                    # so rstd comes from one DVE Newton pass (no ACT Sqrt ->
                    # the Exp LUT never gets evicted).
                    X_h = XB if k == 0 else XA
                    mv4 = lnpool.tile([P, OS, 2], f32, tag="mv4")
                    for osub in range(OS):
                        oc = ot * OS + osub  # node chunk index
                        ps = pspool.tile([P, E], f32, tag="mm")
                        for ec in range(EC):
                            nc.tensor.matmul(
                                ps,
                                lhsT=ctxT[:, ec, osub * P : (osub + 1) * P],
                                rhs=wc_sb[:, ec, :],
                                start=(ec == 0),
                                stop=(ec == EC - 1),
                            )
                        h = X_h[:, oc, :]
                        nc.vector.tensor_add(h, ps, X_in[:, oc, :])
                        stats = lnpool.tile([P, 6], f32, tag="st")
                        nc.vector.bn_stats(stats, h)
                        nc.vector.bn_aggr(mv4[:, osub, :], stats)
                    # rstd4 = 1/sqrt(var4 + eps): magic seed + 2 Newton steps
                    x4 = lnpool.tile([P, OS], f32, tag="x4")
                    y4 = lnpool.tile([P, OS], f32, tag="y4")
                    t4 = lnpool.tile([P, OS], f32, tag="t4")
                    nc.vector.tensor_scalar_add(x4, mv4[:, :, 1], LN_EPS)
                    nc.vector.tensor_scalar(
                        out=y4.bitcast(i32),
                        in0=x4.bitcast(i32),
                        scalar1=1,
                        scalar2=None,
                        op0=ALU.logical_shift_right,
                    )
                    nc.vector.tensor_scalar(
                        out=y4.bitcast(i32),
                        in0=y4.bitcast(i32),
                        scalar1=-1,
                        scalar2=0x5F3759DF,
                        op0=ALU.mult,
                        op1=ALU.add,
                    )
                    for _ in range(2):
                        nc.vector.tensor_mul(t4, y4, y4)
                        nc.vector.tensor_mul(t4, t4, x4)
                        nc.vector.tensor_scalar(
                            out=t4,
                            in0=t4,
                            scalar1=-0.5,
                            scalar2=1.5,
                            op0=ALU.mult,
                            op1=ALU.add,
                        )
                        nc.vector.tensor_mul(y4, y4, t4)
                    for osub in range(OS):
                        oc = ot * OS + osub
                        h = X_h[:, oc, :]
                        nc.vector.tensor_scalar(
                            out=h,
                            in0=h,
                            scalar1=mv4[:, osub, 0:1],
                            scalar2=y4[:, osub : osub + 1],
                            op0=ALU.subtract,
                            op1=ALU.mult,
                        )
                        if apply_gb:
                            nc.gpsimd.tensor_mul(h, h, g_sb[:, k, :])
                            nc.gpsimd.tensor_add(h, h, b_sb[:, k, :])
                        if k == L - 1:
                            nc.sync.dma_start(
                                out.rearrange("(c p) e -> p c e", p=P)[:, oc, :],
                                h.bitcast(f32),
                            )
    nc.compile()
    return nc


def _get_nc(apply_gb: bool):
    key = ("nc", apply_gb)
    if key not in _CACHE:
        _CACHE[key] = _build(apply_gb)
    return _CACHE[key]


def make_in_maps(inputs, apply_gb=None):
    node_fts = np.asarray(inputs["node_fts"], np.float32)
    rel_edges = np.asarray(inputs["rel_edges"])
    Wq = np.asarray(inputs["Wq"], np.float32)
    bq = np.asarray(inputs["bq"], np.float32)
    Wk = np.asarray(inputs["Wk"], np.float32)
    bk = np.asarray(inputs["bk"], np.float32)
    Wc = np.asarray(inputs["Wc"], np.float32)
    ln_g = np.asarray(inputs["ln_g"], np.float32)
    ln_b = np.asarray(inputs["ln_b"], np.float32)
    if apply_gb is None:
        apply_gb = _needs_gb(inputs)

    wq_t = np.ascontiguousarray(np.transpose(Wq, (0, 2, 1)))  # [L, E, FF]
    wk_t = np.ascontiguousarray(np.transpose(Wk, (0, 2, 1)))
    wc_t = np.ascontiguousarray(np.transpose(Wc, (0, 2, 1)))  # [L, E, E]
    idn = np.eye(P, dtype=np.float32)

    in_maps = []
    for c in range(B):
        m = {
            "x0": np.ascontiguousarray(node_fts[c]),
            "x0t": np.ascontiguousarray(node_fts[c].T),
            "maskt": np.ascontiguousarray(
                (rel_edges[c] != 0).T.astype(np.float32)
            ),
            "wq": wq_t,
            "wk": wk_t,
            "wc": wc_t,
            "idn": idn,
            "onesm": np.ones((P, P), dtype=np.float32),
            "bq": bq,
            "bk": bk,
        }
        if apply_gb:
            m["ln_g"] = ln_g
            m["ln_b"] = ln_b
        in_maps.append(m)
    return in_maps


def _needs_gb(inputs):
    g = np.asarray(inputs["ln_g"], np.float32)
    b = np.asarray(inputs["ln_b"], np.float32)
    return not (np.all(g == 1.0) and np.all(b == 0.0))


def kernel(**inputs) -> np.ndarray:
    from concourse.bass_utils import run_bass_kernel_spmd

    apply_gb = _needs_gb(inputs)
    nc = _get_nc(apply_gb)
    in_maps = make_in_maps(inputs, apply_gb)
    res = run_bass_kernel_spmd(nc, in_maps, core_ids=list(range(B)))
    return np.stack([r["out"] for r in res.results], axis=0)


# revision 47
# speedup vs baseline: 1.0681x; 1.0681x over previous
"""GCN encoder (2-layer masked-attention message passing) on 8 Trainium2 cores.

Sharding: data-parallel over batch B=8 -> 1 graph per NeuronCore. Each core
holds the full (small) parameter set and its own [N, N] edge-mask slab.

Per-core algorithm (N=2048 nodes, E=512, FF=256, 2 layers), all on-chip:
  - x kept in SBUF in natural [n, E] layout (residual/LN) and transposed
    [E, n] layout (projection rhs).
  - Q^T/K^T = W x^T computed with E-contraction on partitions.
  - scores are computed TRANSPOSED: sT[i, o] = sum_f K^T[f,i] Q^T[f,o], so the
    unnormalized attention p = mask * exp(sT/16) is already in the [i, o]
    layout needed as the moving operand of the ctx matmul (no O(N^2) on-chip
    transpose). Softmax max-subtraction is skipped: scores are ~N(0, 0.2^2)
    by construction, exp() cannot overflow, and softmax is shift-invariant.
  - denominator: ones[128,128]^T @ p broadcasts column sums to every PSUM
    partition; normalization (x 1/denom) is fused into the ctx PSUM->SBUF
    copy, so it costs no extra DVE pass over the N^2 matrix.
  - ctx^T[d, o] accumulated over node chunks, then ctx2 = ctx @ Wc^T lands
    back in natural [o, e] layout for residual + LayerNorm (bn_stats/bn_aggr,
    rstd via DVE Newton so the ACT Exp LUT is never evicted).
  - all matmul operands use float32r (fp32 bits, PE streams 1 row/cycle
    instead of fp32's 4 -> 4x matmul throughput, ~1.5e-4 rel err).
"""

import os
import sys

for _p in ("/root/.axon_site/_ro/trn_rl_repo", "/opt/trn_rl_repo"):
    if os.path.isdir(_p) and _p not in sys.path:
        sys.path.append(_p)

import numpy as np

B, N, E, FF, L = 8, 2048, 512, 256, 2
P = 128
NC = N // P  # 16 node chunks
EC = E // P  # 4 embed chunks
FC = FF // P  # 2 ff chunks
OW = 512  # o-tile width (out-node tile)
OT = N // OW  # 4 o-tiles
OS = OW // P  # 4 o-subtiles per o-tile
INV_SCALE = 1.0 / float(np.sqrt(FF))
LN_EPS = 1e-5

_CACHE = {}


def _build(apply_gb: bool):
    import concourse.bass as bass
    import concourse.tile as tile
    from concourse import bacc, mybir

    f32 = mybir.dt.float32
    f32r = mybir.dt.float32r
    i32 = mybir.dt.int32
    AF = mybir.ActivationFunctionType
    ALU = mybir.AluOpType

    nc = bacc.Bacc(
        "TRN2", target_bir_lowering=False, debug=False, num_devices=B
    )

    x0 = nc.dram_tensor("x0", [N, E], f32r, kind="ExternalInput")
    x0t = nc.dram_tensor("x0t", [E, N], f32r, kind="ExternalInput")
    # edge mask, transposed: maskt[i, o] = 1.0 if edge(o <- i) else 0.0
    maskt = nc.dram_tensor("maskt", [N, N], f32r, kind="ExternalInput")
    wq = nc.dram_tensor("wq", [L, E, FF], f32r, kind="ExternalInput")  # Wq^T
    wk = nc.dram_tensor("wk", [L, E, FF], f32r, kind="ExternalInput")  # Wk^T
    wc = nc.dram_tensor("wc", [L, E, E], f32r, kind="ExternalInput")  # Wc^T
    idn = nc.dram_tensor("idn", [P, P], f32r, kind="ExternalInput")
    onesm = nc.dram_tensor("onesm", [P, P], f32r, kind="ExternalInput")
    bq = nc.dram_tensor("bq", [L, FF], f32, kind="ExternalInput")
    bk = nc.dram_tensor("bk", [L, FF], f32, kind="ExternalInput")
    if apply_gb:
        ln_g = nc.dram_tensor("ln_g", [L, E], f32, kind="ExternalInput")
        ln_b = nc.dram_tensor("ln_b", [L, E], f32, kind="ExternalInput")
    out = nc.dram_tensor("out", [N, E], f32, kind="ExternalOutput")

    with tile.TileContext(nc) as tc:
        with (
            tc.tile_pool(name="persist", bufs=1) as persist,
            tc.tile_pool(name="wpool", bufs=1) as wpool,
            tc.tile_pool(name="qt", bufs=1) as qtpool,
            tc.tile_pool(name="pt", bufs=1) as ptpool,
            tc.tile_pool(name="ctxt", bufs=1) as ctxtpool,
            tc.tile_pool(name="mask", bufs=6 if not apply_gb else 4) as mpool,
            tc.tile_pool(name="rb", bufs=2) as rbpool,
            tc.tile_pool(name="ln", bufs=8) as lnpool,
            tc.tile_pool(name="ps", bufs=6, space="PSUM") as pspool,
            tc.tile_pool(name="psdn", bufs=2, space="PSUM") as dnpool,
        ):
            # ---------------- persistent tiles ----------------
            XA = persist.tile([P, NC, E], f32r, tag="XA")  # layer-1 input
            XB = persist.tile([P, NC, E], f32r, tag="XB")  # layer-1 output
            XT = persist.tile([P, EC, N], f32r, tag="XT")  # x^T of current layer
            KT = persist.tile([P, FC, N], f32r, tag="KT")
            ident = persist.tile([P, P], f32r, tag="ident")
            ones = persist.tile([P, P], f32r, tag="ones")
            bq_sb = persist.tile([P, L, FC], f32, tag="bq")
            bk_sb = persist.tile([P, L, FC], f32, tag="bk")

            def bcast_ap(src):  # broadcast a DRAM AP across all partitions
                return bass.AP(
                    tensor=src.tensor, offset=src.offset, ap=[[0, P], *src.ap]
                )

            # DMA issue order matters: HWDGE queues drain in issue order, so
            # put the first-consumed tensors (layer-0 K/Q weights, XT) first
            # and route the small constants through the SWDGE (gpsimd) queue.
            nc.gpsimd.dma_start(out=ident, in_=idn[:, :])
            nc.gpsimd.dma_start(out=ones, in_=onesm[:, :])
            nc.gpsimd.dma_start(
                out=bq_sb, in_=bq.rearrange("l (c p) -> p l c", p=P)
            )
            nc.gpsimd.dma_start(
                out=bk_sb, in_=bk.rearrange("l (c p) -> p l c", p=P)
            )
            g_sb = b_sb = None
            if apply_gb:
                g_sb = persist.tile([P, L, E], f32, tag="g")
                b_sb = persist.tile([P, L, E], f32, tag="b")
                nc.gpsimd.dma_start(out=g_sb, in_=bcast_ap(ln_g[:, :]))
                nc.gpsimd.dma_start(out=b_sb, in_=bcast_ap(ln_b[:, :]))

            def load_weights(k, split=False):
                wq_sb = wpool.tile([P, EC, FF], f32r, tag="wq")
                wk_sb = wpool.tile([P, EC, FF], f32r, tag="wk")
                wc_sb = wpool.tile([P, EC, E], f32r, tag="wc")
                nc.sync.dma_start(
                    out=wk_sb, in_=wk[k].rearrange("(c p) f -> p c f", p=P)
                )
                if not split:
                    nc.sync.dma_start(
                        out=wq_sb, in_=wq[k].rearrange("(c p) f -> p c f", p=P)
                    )
                    nc.sync.dma_start(
                        out=wc_sb, in_=wc[k].rearrange("(c p) e -> p c e", p=P)
                    )
                return wq_sb, wk_sb, wc_sb

            # issue order: wk -> XT(nt0) -> rest of XT -> wq -> wc, so the
            # first K-projection matmul starts after ~1.5MB instead of 10MB
            w0 = load_weights(0, split=True)
            x0tr = x0t.rearrange("(c p) n -> p c n", p=P)
            for nt in range(OT):
                for ec in range(EC):
                    nc.sync.dma_start(
                        out=XT[:, ec, nt * OW : (nt + 1) * OW],
                        in_=x0tr[:, ec, nt * OW : (nt + 1) * OW],
                    )
                if nt == 0:
                    nc.sync.dma_start(
                        out=w0[0], in_=wq[0].rearrange("(c p) f -> p c f", p=P)
                    )
            nc.sync.dma_start(
                out=w0[2], in_=wc[0].rearrange("(c p) e -> p c e", p=P)
            )
            x0r = x0.rearrange("(c p) e -> p c e", p=P)

            def emit_transposes(ot_src, X_src):
                # transpose o-tile ot_src's normed chunks into XT for the
                # next layer (XT is dead once the current layer's projections
                # are done). Four 128x128 transposes land in disjoint
                # quarters of a 512-wide PSUM tile, drained by one wide ACT
                # copy. Emission is deferred until the source LN chain has
                # finished so PE never stalls on it.
                for ec in range(EC):
                    pst = pspool.tile([P, 4 * P], f32r, tag="mm")
                    for j in range(OS):
                        oc = ot_src * OS + j
                        nc.tensor.matmul(
                            pst[:, j * P : (j + 1) * P],
                            lhsT=X_src[:, oc, ec * P : (ec + 1) * P],
                            rhs=ident,
                            is_transpose=True,
                            start=True,
                            stop=True,
                            skip_group_check=True,
                        )
                    dst = XT[:, ec, ot_src * OW : (ot_src + 1) * OW]
                    nc.scalar.copy(dst, pst)

            pending_tr = None
            w_next = w0
            for k in range(L):
                X_in = XA if k == 0 else XB

                wq_sb, wk_sb, wc_sb = w_next

                # K^T and Q^T projections for the whole layer
                QT = qtpool.tile([P, FC, N], f32r, tag="qt")
                for dst, w_sb, b_sb2 in ((KT, wk_sb, bk_sb), (QT, wq_sb, bq_sb)):
                    for nt in range(OT):
                        if nt == OT - 1 and pending_tr is not None:
                            # last XT slab: flush the deferred transposes of
                            # the previous layer's final o-tile first
                            emit_transposes(*pending_tr)
                            pending_tr = None
                        for fc in range(FC):
                            ps = pspool.tile([P, OW], f32, tag="mm")
                            for ec in range(EC):
                                nc.tensor.matmul(
                                    ps,
                                    lhsT=w_sb[:, ec, fc * P : (fc + 1) * P],
                                    rhs=XT[:, ec, nt * OW : (nt + 1) * OW],
                                    start=(ec == 0),
                                    stop=(ec == EC - 1),
                                )
                            nc.scalar.add(
                                dst[:, fc, nt * OW : (nt + 1) * OW],
                                ps,
                                b_sb2[:, k, fc : fc + 1],
                            )

                # queue next layer's weight loads now: wk/wq slots are free
                # after the projections above, so the DMAs overlap this
                # layer's attention phases instead of stalling layer k+1
                if k + 1 < L:
                    w_next = load_weights(k + 1)

                for ot in range(OT):
                    osl = slice(ot * OW, (ot + 1) * OW)
                    QTt = QT[:, :, osl]

                    # scores^T + exp + mask -> pT[i, o]; denom via ones-matmul
                    pT = ptpool.tile([P, NC, OW], f32r, tag="pT")
                    dn = dnpool.tile([P, OW], f32, tag="dn")
                    for ic in range(NC):
                        ps = pspool.tile([P, OW], f32, tag="mm")
                        for fc in range(FC):
                            nc.tensor.matmul(
                                ps,
                                lhsT=KT[:, fc, ic * P : (ic + 1) * P],
                                rhs=QTt[:, fc, :],
                                start=(fc == 0),
                                stop=(fc == FC - 1),
                            )
                        nc.scalar.activation(
                            pT[:, ic, :], ps, AF.Exp, scale=INV_SCALE
                        )
                        mt = mpool.tile([P, OW], f32r, tag="mt")
                        nc.sync.dma_start(
                            mt, maskt[ic * P : (ic + 1) * P, osl]
                        )
                        nc.vector.tensor_mul(pT[:, ic, :], pT[:, ic, :], mt)
                        # dn[p, o] = column sums of p, broadcast to all partitions
                        nc.tensor.matmul(
                            dn,
                            lhsT=ones,
                            rhs=pT[:, ic, :],
                            start=(ic == 0),
                            stop=(ic == NC - 1),
                        )
                        if k == 0 and ot == 0:
                            # stream layer-0 x in behind the first masks;
                            # it is first needed by phase C of o-tile 0
                            nc.sync.dma_start(
                                out=XA[:, ic, :], in_=x0r[:, ic, :]
                            )
                    rb = rbpool.tile([P, OW], f32, tag="rb")
                    nc.vector.reciprocal(rb, dn)

                    # ctx^T[d, o] = sum_i x[i, d] * p[i, o], normalized by rb
                    ctxT = ctxtpool.tile([P, EC, OW], f32r, tag="ctxT")
                    for ec in range(EC):
                        ps = pspool.tile([P, OW], f32, tag="mm")
                        for ic in range(NC):
                            nc.tensor.matmul(
                                ps,
                                lhsT=X_in[:, ic, ec * P : (ec + 1) * P],
                                rhs=pT[:, ic, :],
                                start=(ic == 0),
                                stop=(ic == NC - 1),
                            )
                        nc.vector.tensor_mul(ctxT[:, ec, :], ps, rb)
                    if k < L - 1 and ot > 0:
                        emit_transposes(ot - 1, XB if k == 0 else XA)

                    # ctx2 = ctx @ Wc^T, residual, LayerNorm.
                    # Layer-1 output lives in XB; layer-2 h reuses XA (dead
                    # in layer 2). LN stats for the 4 o-subtiles are batched
                    # so rstd comes from one DVE Newton pass (no ACT Sqrt ->
                    # the Exp LUT never gets evicted).
                    X_h = XB if k == 0 else XA
                    mv4 = lnpool.tile([P, OS, 2], f32, tag="mv4")
                    for osub in range(OS):
                        oc = ot * OS + osub  # node chunk index
                        ps = pspool.tile([P, E], f32, tag="mm")
                        for ec in range(EC):
                            nc.tensor.matmul(
                                ps,
                                lhsT=ctxT[:, ec, osub * P : (osub + 1) * P],
                                rhs=wc_sb[:, ec, :],
                                start=(ec == 0),
                                stop=(ec == EC - 1),
                            )
                        h = X_h[:, oc, :]
                        nc.vector.tensor_add(h, ps, X_in[:, oc, :])
                        stats = lnpool.tile([P, 6], f32, tag="st")
                        nc.vector.bn_stats(stats, h)
                        nc.vector.bn_aggr(mv4[:, osub, :], stats)
                    # rstd4 = 1/sqrt(var4 + eps): magic seed + 2 Newton steps
                    x4 = lnpool.tile([P, OS], f32, tag="x4")
                    y4 = lnpool.tile([P, OS], f32, tag="y4")
                    t4 = lnpool.tile([P, OS], f32, tag="t4")
                    nc.vector.tensor_scalar_add(x4, mv4[:, :, 1], LN_EPS)
                    nc.vector.tensor_scalar(
                        out=y4.bitcast(i32),
                        in0=x4.bitcast(i32),
                        scalar1=1,
                        scalar2=None,
                        op0=ALU.logical_shift_right,
                    )
                    nc.vector.tensor_scalar(
                        out=y4.bitcast(i32),
                        in0=y4.bitcast(i32),
                        scalar1=-1,
                        scalar2=0x5F3759DF,
                        op0=ALU.mult,
                        op1=ALU.add,
                    )
                    for _ in range(2):
                        nc.vector.tensor_mul(t4, y4, y4)
                        nc.vector.tensor_mul(t4, t4, x4)
                        nc.vector.tensor_scalar(
                            out=t4,
                            in0=t4,
                            scalar1=-0.5,
                            scalar2=1.5,
                            op0=ALU.mult,
                            op1=ALU.add,
                        )
                        nc.vector.tensor_mul(y4, y4, t4)
                    for osub in range(OS):
                        oc = ot * OS + osub
                        h = X_h[:, oc, :]
                        nc.vector.tensor_scalar(
                            out=h,
                            in0=h,
                            scalar1=mv4[:, osub, 0:1],
                            scalar2=y4[:, osub : osub + 1],
                            op0=ALU.subtract,
                            op1=ALU.mult,
                        )
                        if apply_gb:
                            nc.gpsimd.tensor_mul(h, h, g_sb[:, k, :])
                            nc.gpsimd.tensor_add(h, h, b_sb[:, k, :])
                        if k == L - 1:
                            nc.sync.dma_start(
                                out.rearrange("(c p) e -> p c e", p=P)[:, oc, :],
                                h.bitcast(f32),
                            )
                if k < L - 1:
                    pending_tr = (OT - 1, XB if k == 0 else XA)
    nc.compile()
    return nc


def _get_nc(apply_gb: bool):
    key = ("nc", apply_gb)
    if key not in _CACHE:
        _CACHE[key] = _build(apply_gb)
    return _CACHE[key]


def make_in_maps(inputs, apply_gb=None):
    node_fts = np.asarray(inputs["node_fts"], np.float32)
    rel_edges = np.asarray(inputs["rel_edges"])
    Wq = np.asarray(inputs["Wq"], np.float32)
    bq = np.asarray(inputs["bq"], np.float32)
    Wk = np.asarray(inputs["Wk"], np.float32)
    bk = np.asarray(inputs["bk"], np.float32)
    Wc = np.asarray(inputs["Wc"], np.float32)
    ln_g = np.asarray(inputs["ln_g"], np.float32)
    ln_b = np.asarray(inputs["ln_b"], np.float32)
    if apply_gb is None:
        apply_gb = _needs_gb(inputs)

    wq_t = np.ascontiguousarray(np.transpose(Wq, (0, 2, 1)))  # [L, E, FF]
    wk_t = np.ascontiguousarray(np.transpose(Wk, (0, 2, 1)))
    wc_t = np.ascontiguousarray(np.transpose(Wc, (0, 2, 1)))  # [L, E, E]
    idn = np.eye(P, dtype=np.float32)

    in_maps = []
    for c in range(B):
        m = {
            "x0": np.ascontiguousarray(node_fts[c]),
            "x0t": np.ascontiguousarray(node_fts[c].T),
            "maskt": np.ascontiguousarray(
                (rel_edges[c] != 0).T.astype(np.float32)
            ),
            "wq": wq_t,
            "wk": wk_t,
            "wc": wc_t,
            "idn": idn,
            "onesm": np.ones((P, P), dtype=np.float32),
            "bq": bq,
            "bk": bk,
        }
        if apply_gb:
            m["ln_g"] = ln_g
            m["ln_b"] = ln_b
        in_maps.append(m)
    return in_maps


def _needs_gb(inputs):
    g = np.asarray(inputs["ln_g"], np.float32)
    b = np.asarray(inputs["ln_b"], np.float32)
    return not (np.all(g == 1.0) and np.all(b == 0.0))


def kernel(**inputs) -> np.ndarray:
    from concourse.bass_utils import run_bass_kernel_spmd

    apply_gb = _needs_gb(inputs)
    nc = _get_nc(apply_gb)
    in_maps = make_in_maps(inputs, apply_gb)
    res = run_bass_kernel_spmd(nc, in_maps, core_ids=list(range(B)))
    return np.stack([r["out"] for r in res.results], axis=0)
